# revision 17
# baseline (speedup 1.0000x reference)
"""BalancedMoE (B=8192, D=2048, E=8, top-2) on 8 Trainium2 NeuronCores.

Strategy: expert-parallel with host-side sparse dispatch.
  - Host computes gate logits / top-2 routing / softmax gates, gathers each
    expert's tokens into a partition-major [P, KT, C] bf16 layout, and
    pre-permutes the expert weight into [P, MT, KT, P] bf16.
  - Core e runs a dense [D, D] x [D, C] matmul for expert e (top-2 of 8
    experts => 4x less FLOPs than the dense reference). bf16 runs the PE at
    the same 1 column/cycle as fp32r but halves the input DMA footprint, so
    the whole working set stays SBUF-resident.
  - DMA: per-descriptor overhead (~300ns) dominates small transfers, so every
    DMA moves multi-KB contiguous runs per partition: weights ride the
    Activation queue in 5 grouped chunks (4..16KB runs), tokens ride the Sync
    queue as 2 pair + 3 quad k-slices (9..18KB runs), outputs (bf16) ride the
    GpSimd SWDGE queue as per-row halves (~4.5KB runs).
  - Startup: the first rows are processed in k-eighths/quarters (separate
    PSUM accumulation groups combined via f32 partials in SBUF) so the
    in-order PE queue chases the token k-front instead of head-of-line
    blocking; remaining rows run full-k accumulation (j-outer, k-inner:
    back-to-back same-bank accumulate keeps the PE pipeline full).

Per-core Bass kernel: outT[o, t] = sum_d W_e[o, d] * toks[t, d] + b_e[o]
"""

import os

import numpy as np

P = 128
B = 8192
D_LAT = 1024
D_EMB = 1024
D = D_LAT + D_EMB  # 2048
E = 8
TOPK = 2
N_CORES = 8
KT = D // P  # 16
MT = D // P  # 16
NQ = 3  # startup m-rows processed in k-quarters (m0 in k-eighths for k0..3)

# m-chunk DMA groups: group 0 rides the Sync queue ahead of the tokens (that
# queue opens ~3us earlier); 1..3 are singles so the quartered rows m1/m2 and
# the first full row never wait on a fat weight transfer.
W_GROUPS = [(0, 1), (1, 2), (2, 3), (3, 4), (4, 8), (8, 12), (12, 16)]
T_GROUPS = [(0, 2), (2, 4), (4, 8), (8, 12), (12, 16)]  # k-slice DMA groups


# ----------------------------------------------------------------- device ---

_cache = {}


def _ntff_shim():
    """Register the axon NTFF profile hook that the boot skips when
    antenv.axon_hooks is missing (so BASS_TRACE=1 yields exec_time_ns)."""
    import sys
    import types

    if "antenv.axon_hooks" in sys.modules:
        return
    holder = [None]
    mod = types.ModuleType("antenv.axon_hooks")
    mod.set_axon_ntff_profile_hook = lambda h: holder.__setitem__(0, h)
    mod.get_axon_ntff_profile_hook = lambda: holder[0]
    sys.modules["antenv.axon_hooks"] = mod
    try:
        import antenv

        antenv.axon_hooks = mod
        from trn_agent_boot.trn_boot import _ntff_profile_via_ctypes

        mod.set_axon_ntff_profile_hook(
            _ntff_profile_via_ctypes("/opt/axon/libaxon_pjrt.so")
        )
    except Exception:
        pass


def _n_tiles(C):
    """Split C into moving-operand tiles of width 256..512 (>=256 columns per
    matmul keeps the PE at full rate; PSUM caps a tile at 512)."""
    assert C >= 512
    k = (C - 256) // 512 if C % 512 else C // 512
    rem = C - 512 * k
    sizes = [512] * k
    if rem == 0:
        pass
    elif rem <= 512:
        sizes.append(rem)
    else:  # 513..767: two tiles, both >= 256
        sizes.extend([rem - 256, 256])
    return sizes


def _build(C):
    import concourse.mybir as mybir
    from concourse import bacc
    from concourse.bass import ds
    from concourse.tile import TileContext

    dt = mybir.dt.bfloat16
    f32 = mybir.dt.float32
    n_sizes = _n_tiles(C)
    J = len(n_sizes)
    n_offs = [0] * J
    for j in range(1, J):
        n_offs[j] = n_offs[j - 1] + n_sizes[j - 1]

    nc = bacc.Bacc(
        "TRN2", target_bir_lowering=False, debug=False, num_devices=N_CORES
    )
    # wpt[ki, m, ko, o] = W_e[m*128 + o, ko*128 + ki] — partition-major so a
    # group of m-chunks is one DMA with mg*4KB contiguous runs per partition.
    wpt = nc.dram_tensor("wpt", [P, MT, KT, P], dt, kind="ExternalInput")
    # tokq[ki, k, c] = inp[idx[c], k*128 + ki] — partition-major so a group of
    # k-slices is one DMA with ng*C*2 contiguous runs per partition.
    tokq = nc.dram_tensor("tokq", [P, KT, C], dt, kind="ExternalInput")
    # bias_t[p, m] = b[m*128 + p]: pre-transposed on host so the DMA reads
    # contiguous 64B runs instead of a 4-byte-per-descriptor gather.
    bias = nc.dram_tensor("bias_t", [P, MT], f32, kind="ExternalInput")
    outT = nc.dram_tensor("outT", [MT, P, C], dt, kind="ExternalOutput")

    nq = min(NQ, MT)

    with TileContext(nc) as tc:
        with (
            tc.tile_pool(name="w", bufs=1) as w_pool,
            tc.tile_pool(name="tok", bufs=1) as tok_pool,
            tc.tile_pool(name="acc", bufs=1) as acc_pool,
            tc.tile_pool(name="orow", bufs=6) as orow_pool,
            tc.tile_pool(name="bias", bufs=1) as b_pool,
            tc.tile_pool(name="ps", bufs=8, space="PSUM") as ps_pool,
        ):
            bias_tile = b_pool.tile([P, MT], f32)

            # ---- input DMAs: w-group0 + tokens on the Sync queue, bias +
            # remaining weight groups on the Activation queue.
            w_tiles = []  # one tile per W_GROUP
            for lo, hi in W_GROUPS:
                w_tiles.append(
                    w_pool.tile([P, hi - lo, KT, P], dt, tag=f"w{lo}", name=f"w{lo}")
                )
            lo, hi = W_GROUPS[0]
            nc.sync.dma_start(w_tiles[0][:], wpt.ap()[:, ds(lo, hi - lo)])
            tok_tiles = []  # one tile per T_GROUP
            for lo, hi in T_GROUPS:
                t = tok_pool.tile([P, hi - lo, C], dt, tag=f"t{lo}")
                nc.sync.dma_start(t[:], tokq.ap()[:, ds(lo, hi - lo)])
                tok_tiles.append(t)
            nc.scalar.dma_start(bias_tile[:], bias.ap())
            for gi, (lo, hi) in enumerate(W_GROUPS):
                if gi == 0:
                    continue
                nc.scalar.dma_start(w_tiles[gi][:], wpt.ap()[:, ds(lo, hi - lo)])

            # ---- PE warmup: ~14 matmuls on a zeroed const tile lift the PE
            # out of the low/mid p-state (2x clock) before real operands land.
            warm = b_pool.tile([P, 512], dt, tag="warm")
            nc.vector.memset(warm[:], 0)
            wps = ps_pool.tile([P, 512], f32, tag="ps")
            for i in range(14):
                nc.tensor.matmul(
                    wps, warm[:, :128], warm[:], start=(i == 0), stop=(i == 13)
                )
            nc.vector.tensor_copy(warm[:], wps)

            def rhs(k, j):
                for gi, (lo, hi) in enumerate(T_GROUPS):
                    if k < hi:
                        return tok_tiles[gi][:, k - lo, ds(n_offs[j], n_sizes[j])]
                raise AssertionError

            def lhsT(m, k):
                for gi, (lo, hi) in enumerate(W_GROUPS):
                    if m < hi:
                        return w_tiles[gi][:, m - lo, k, :]
                raise AssertionError

            # output row DMA in halves: half A fires mid-row (after tile ja's
            # drain), so only the short half B trails the last matmul
            ja = 1 if J >= 2 else 0
            h_split = n_offs[ja] + n_sizes[ja]

            def out_dma_a(m, orow):
                nc.gpsimd.dma_start(
                    outT.ap()[m][:, ds(0, h_split)], orow[:, ds(0, h_split)]
                )

            def out_dma_b(m, orow):
                if C > h_split:
                    nc.gpsimd.dma_start(
                        outT.ap()[m][:, ds(h_split, C - h_split)],
                        orow[:, ds(h_split, C - h_split)],
                    )

            # ---- startup: first nq rows in k-chunks chasing the token front.
            # m0 row: k-chunks [0,1],[2,3],[4..7],[8..11],[12..15]
            # m1/m2  : k-chunks [0..3],[4..7],[8..11],[12..15]
            chunks = {0: T_GROUPS}
            for m in range(1, nq):
                chunks[m] = [(0, 4), (4, 8), (8, 12), (12, 16)]
            n_chunks = {m: len(chunks[m]) for m in chunks}
            acc_tiles = {}
            orow_q = {}
            # arrival-chasing order: a row emits its k-chunk in the phase of
            # the token group that completes that chunk's k-range
            emit = []  # (m, chunk_index)
            for ci, (tlo, thi) in enumerate(T_GROUPS):
                for m in range(nq):
                    for qi, (klo, khi) in enumerate(chunks[m]):
                        if tlo < khi <= thi:
                            emit.append((m, qi))
            for m, qi in emit:
                klo, khi = chunks[m][qi]
                last = qi == n_chunks[m] - 1
                for j in range(J):
                    psf = ps_pool.tile([P, 512], f32, tag="ps")
                    pj = psf[:, : n_sizes[j]]
                    for k in range(klo, khi):
                        nc.tensor.matmul(
                            pj,
                            lhsT(m, k),
                            rhs(k, j),
                            start=(k == klo),
                            stop=(k == khi - 1),
                        )
                    if qi == 0:
                        a_full = acc_pool.tile([P, 512], f32, tag=f"acc{m}_{j}")
                        a = a_full[:, : n_sizes[j]]
                        acc_tiles[(m, j)] = a
                        # ACT engine: a = ps + bias (keeps DVE free)
                        nc.scalar.add(a, pj, bias_tile[:, m : m + 1])
                    elif not last:
                        a = acc_tiles[(m, j)]
                        nc.vector.tensor_add(a, a, pj)
                    else:
                        if m not in orow_q:
                            orow_q[m] = orow_pool.tile(
                                [P, C], dt, tag="orow", name=f"orow_q{m}"
                            )
                        o = orow_q[m][:, ds(n_offs[j], n_sizes[j])]
                        a = acc_tiles[(m, j)]
                        nc.vector.tensor_add(o, a, pj)
                        if j == ja:
                            out_dma_a(m, orow_q[m])
                if last:
                    out_dma_b(m, orow_q[m])

            # ---- steady state: full-k rows, j-outer k-inner (back-to-back
            # same-bank accumulate keeps the PE pipeline full).
            for m in range(nq, MT):
                orow = orow_pool.tile([P, C], dt, tag="orow")
                for j in range(J):
                    psf = ps_pool.tile([P, 512], f32, tag="ps")
                    pj = psf[:, : n_sizes[j]]
                    for k in range(KT):
                        nc.tensor.matmul(
                            pj,
                            lhsT(m, k),
                            rhs(k, j),
                            start=(k == 0),
                            stop=(k == KT - 1),
                        )
                    o = orow[:, ds(n_offs[j], n_sizes[j])]
                    nc.vector.tensor_scalar_add(o, pj, bias_tile[:, m : m + 1])
                    if j == ja:
                        out_dma_a(m, orow)
                out_dma_b(m, orow)
    nc.compile()
    return nc


def _get_program(C):
    if C not in _cache:
        _cache[C] = _build(C)
    return _cache[C]


# ------------------------------------------------------------------- host ---


def kernel(x, y, W_experts, b_experts, W_gate, b_gate):
    import ml_dtypes

    bf16 = np.dtype(ml_dtypes.bfloat16)

    x = np.asarray(x, dtype=np.float32)
    y = np.asarray(y, dtype=np.float32)
    W_experts = np.asarray(W_experts, dtype=np.float32)
    b_experts = np.asarray(b_experts, dtype=np.float32)
    W_gate = np.asarray(W_gate, dtype=np.float32)
    b_gate = np.asarray(b_gate, dtype=np.float32)

    inp = np.concatenate([x, y], axis=1)  # [B, D]

    # ---- routing (host) ----
    logits = inp.astype(np.float64) @ W_gate.T.astype(np.float64) + b_gate
    order = np.argsort(-logits, axis=1, kind="stable")
    top2 = order[:, :TOPK]  # [B, 2]
    v = np.take_along_axis(logits, top2, axis=1)
    v = v - v.max(axis=1, keepdims=True)
    ev = np.exp(v)
    g = (ev / ev.sum(axis=1, keepdims=True)).astype(np.float32)  # [B, 2]

    counts = np.bincount(top2.ravel(), minlength=E)
    C = max(512, int(counts.max()))

    idx_list = []
    wgt_list = []
    for e in range(E):
        m0 = top2[:, 0] == e
        m1 = top2[:, 1] == e
        idx_e = np.concatenate([np.nonzero(m0)[0], np.nonzero(m1)[0]])
        w_e = np.concatenate([g[m0, 0], g[m1, 1]])
        idx_list.append(idx_e)
        wgt_list.append(w_e)

    inp_bf = inp.astype(bf16)  # [B, D]
    in_maps = []
    for e in range(E):
        n_e = len(idx_list[e])
        # tokq[p, k, c] = inp[idx[c], k*128 + p]
        tokq = np.zeros((P, KT, C), dtype=bf16)
        tokq[:, :, :n_e] = (
            inp_bf[idx_list[e]].T.reshape(KT, P, n_e).transpose(1, 0, 2)
        )
        # wpt[ki, m, ko, o] = W_e[m*128 + o, ko*128 + ki]
        wpt = np.ascontiguousarray(
            W_experts[e].reshape(MT, P, KT, P).transpose(3, 0, 2, 1).astype(bf16)
        )
        bias_t = np.ascontiguousarray(b_experts[e].reshape(MT, P).T)
        in_maps.append({"wpt": wpt, "tokq": tokq, "bias_t": bias_t})

    # ---- device ----
    if os.environ.get("BASS_TRACE"):
        _ntff_shim()
    from concourse.bass_utils import run_bass_kernel_spmd

    nc = _get_program(C)
    res = None
    for attempt in range(3):
        try:
            res = run_bass_kernel_spmd(nc, in_maps, core_ids=list(range(N_CORES)))
            break
        except Exception:
            # the axon-tunneled device occasionally reports a transient
            # NRT_EXEC_UNIT_UNRECOVERABLE; it recovers after a short wait
            if attempt == 2:
                raise
            import time

            time.sleep(20 * (attempt + 1))
            try:
                import jax

                jax.clear_caches()
            except Exception:
                pass
    globals()["_last_res"] = res
    if res.exec_time_ns is not None:
        print(f"HW exec time: {res.exec_time_ns} ns")

    # ---- combine (host) ----
    fused = np.zeros((B, D), dtype=np.float32)
    for e in range(E):
        n_e = len(idx_list[e])
        if n_e == 0:
            continue
        out_rows = (
            res.results[e]["outT"].reshape(D, C)[:, :n_e].T.astype(np.float32)
        )
        fused[idx_list[e]] += out_rows * wgt_list[e][:, None]
    return fused


# revision 18
# speedup vs baseline: 1.1276x; 1.1276x over previous
"""BalancedMoE (B=8192, D=2048, E=8, top-2) on 8 Trainium2 NeuronCores.

Strategy: expert-parallel with host-side sparse dispatch and K-SPLIT expert
pairing to balance the cores.

  - Host computes gate logits / top-2 routing / softmax gates and gathers
    each expert's tokens.  Each expert's GEMM is split into two k-halves
    (d < 1024 and d >= 1024).  Every core runs two jobs: slot A = one
    k-half of one of the 4 LARGEST experts (padded to the static C_A =
    max big count), slot B = one k-half of one of the 4 smallest (padded
    to C_B).  The two halves of an expert land on different cores; the
    host sums the two bf16 partials, adds the bias, and applies the gate
    weights.  Per-core PE work drops from 16*max(c_e) k-columns to
    8*(C_A + C_B) — within ~4% of perfect balance.
  - bf16 inputs run the PE at the same 1 column/cycle as fp32r but halve
    the DMA footprint, so the whole working set stays SBUF-resident.
  - DMA: per-descriptor overhead (~300ns) dominates small transfers, so
    every transfer moves multi-KB contiguous runs per partition
    (partition-major layouts).  Tokens + first weight chunk ride the Sync
    queue, remaining weights the Activation queue, outputs the GpSimd
    SWDGE queue.
  - Startup: the first rows of job A are processed in k-chunks (separate
    PSUM accumulation groups combined via f32 partials in SBUF) so the
    in-order PE queue chases the token arrival front; all later rows run
    full-k accumulation (j-outer, k-inner keeps the PE pipeline full).
"""

import os

import numpy as np

P = 128
B = 8192
D_LAT = 1024
D_EMB = 1024
D = D_LAT + D_EMB  # 2048
E = 8
TOPK = 2
N_CORES = 8
KT = D // P  # 16
KH = KT // 2  # k-tiles per half-job = 8
MT = D // P  # 16
NQ = 3  # startup m-rows of job A processed in k-chunks

# weight m-chunk DMA groups for job A: group 0 rides the Sync queue ahead of
# the tokens; 1..3 are singles so early rows never wait on a fat transfer.
WA_GROUPS = [(0, 1), (1, 2), (2, 3), (3, 4), (4, 8), (8, 12), (12, 16)]
WB_GROUPS = [(0, 4), (4, 8), (8, 12), (12, 16)]
TA_GROUPS = [(0, 2), (2, 4), (4, 8)]  # k-slice groups of job A (8 k-tiles)
TB_GROUPS = [(0, 4), (4, 8)]


# ----------------------------------------------------------------- device ---

_cache = {}


def _ntff_shim():
    """Register the axon NTFF profile hook that the boot skips when
    antenv.axon_hooks is missing (so BASS_TRACE=1 yields exec_time_ns)."""
    import sys
    import types

    if "antenv.axon_hooks" in sys.modules:
        return
    holder = [None]
    mod = types.ModuleType("antenv.axon_hooks")
    mod.set_axon_ntff_profile_hook = lambda h: holder.__setitem__(0, h)
    mod.get_axon_ntff_profile_hook = lambda: holder[0]
    sys.modules["antenv.axon_hooks"] = mod
    try:
        import antenv

        antenv.axon_hooks = mod
        from trn_agent_boot.trn_boot import _ntff_profile_via_ctypes

        mod.set_axon_ntff_profile_hook(
            _ntff_profile_via_ctypes("/opt/axon/libaxon_pjrt.so")
        )
    except Exception:
        pass


def _n_tiles(C):
    """Split C into moving-operand tiles of width 256..512 (>=256 columns per
    matmul keeps the PE at full rate; PSUM caps a tile at 512)."""
    assert C >= 512
    k = (C - 256) // 512 if C % 512 else C // 512
    rem = C - 512 * k
    sizes = [512] * k
    if rem == 0:
        pass
    elif rem <= 512:
        sizes.append(rem)
    else:  # 513..767: two tiles, both >= 256
        sizes.extend([rem - 256, 256])
    return sizes


def _build(CA, CB):
    import concourse.mybir as mybir
    from concourse import bacc
    from concourse.bass import ds
    from concourse.tile import TileContext

    dt = mybir.dt.bfloat16
    f32 = mybir.dt.float32

    def tiles_of(C):
        sizes = _n_tiles(C)
        offs = [0] * len(sizes)
        for j in range(1, len(sizes)):
            offs[j] = offs[j - 1] + sizes[j - 1]
        return sizes, offs

    a_sizes, a_offs = tiles_of(CA)
    b_sizes, b_offs = tiles_of(CB)

    nc = bacc.Bacc(
        "TRN2", target_bir_lowering=False, debug=False, num_devices=N_CORES
    )
    # w[ki, m, kl, o] = Whalf[m*128 + o, kl*128 + ki]  (partition-major)
    wa = nc.dram_tensor("wa", [P, MT, KH, P], dt, kind="ExternalInput")
    wb = nc.dram_tensor("wb", [P, MT, KH, P], dt, kind="ExternalInput")
    # t[ki, kl, c] = inp[idx[c], (h*8 + kl)*128 + ki]  (partition-major)
    ta = nc.dram_tensor("ta", [P, KH, CA], dt, kind="ExternalInput")
    tb = nc.dram_tensor("tb", [P, KH, CB], dt, kind="ExternalInput")
    outa = nc.dram_tensor("outa", [MT, P, CA], dt, kind="ExternalOutput")
    outb = nc.dram_tensor("outb", [MT, P, CB], dt, kind="ExternalOutput")

    nq = min(NQ, MT)

    with TileContext(nc) as tc:
        with (
            tc.tile_pool(name="w", bufs=1) as w_pool,
            tc.tile_pool(name="tok", bufs=1) as tok_pool,
            tc.tile_pool(name="acc", bufs=1) as acc_pool,
            tc.tile_pool(name="orow", bufs=6) as orow_pool,
            tc.tile_pool(name="warm", bufs=1) as warm_pool,
            tc.tile_pool(name="ps", bufs=8, space="PSUM") as ps_pool,
        ):
            # ---- input DMAs ----
            wa_tiles = []
            for lo, hi in WA_GROUPS:
                wa_tiles.append(
                    w_pool.tile([P, hi - lo, KH, P], dt, tag=f"wa{lo}",
                                name=f"wa{lo}")
                )
            wb_tiles = []
            for lo, hi in WB_GROUPS:
                wb_tiles.append(
                    w_pool.tile([P, hi - lo, KH, P], dt, tag=f"wb{lo}",
                                name=f"wb{lo}")
                )
            # Sync queue: wa group 0, then job-A tokens, then job-B tokens.
            lo, hi = WA_GROUPS[0]
            nc.sync.dma_start(wa_tiles[0][:], wa.ap()[:, ds(lo, hi - lo)])
            ta_tiles = []
            for lo, hi in TA_GROUPS:
                t = tok_pool.tile([P, hi - lo, CA], dt, tag=f"ta{lo}",
                                  name=f"ta{lo}")
                nc.sync.dma_start(t[:], ta.ap()[:, ds(lo, hi - lo)])
                ta_tiles.append(t)
            tb_tiles = []
            for lo, hi in TB_GROUPS:
                t = tok_pool.tile([P, hi - lo, CB], dt, tag=f"tb{lo}",
                                  name=f"tb{lo}")
                nc.sync.dma_start(t[:], tb.ap()[:, ds(lo, hi - lo)])
                tb_tiles.append(t)
            # Activation queue: remaining wa groups, then wb groups.
            for gi, (lo, hi) in enumerate(WA_GROUPS):
                if gi == 0:
                    continue
                nc.scalar.dma_start(
                    wa_tiles[gi][:], wa.ap()[:, ds(lo, hi - lo)]
                )
            for gi, (lo, hi) in enumerate(WB_GROUPS):
                nc.scalar.dma_start(
                    wb_tiles[gi][:], wb.ap()[:, ds(lo, hi - lo)]
                )

            def lhs(groups, tiles, m, k):
                for gi, (lo, hi) in enumerate(groups):
                    if m < hi:
                        return tiles[gi][:, m - lo, k, :]
                raise AssertionError

            def rhs(groups, tiles, offs, sizes, k, j):
                for gi, (lo, hi) in enumerate(groups):
                    if k < hi:
                        return tiles[gi][:, k - lo, ds(offs[j], sizes[j])]
                raise AssertionError

            # ---- PE warmup: lift the PE out of the low p-state before real
            # operands land.
            warm = warm_pool.tile([P, 512], dt)
            nc.vector.memset(warm[:], 0)
            wps = ps_pool.tile([P, 512], f32, tag="ps")
            for i in range(14):
                nc.tensor.matmul(
                    wps, warm[:, :128], warm[:], start=(i == 0), stop=(i == 13)
                )
            nc.vector.tensor_copy(warm[:], wps)

            # output row DMA in halves: half A fires mid-row, so only a short
            # transfer trails the row's last drain
            def make_out_dmas(out_dram, sizes, offs, C):
                ja = 1 if len(sizes) >= 2 else 0
                h_split = offs[ja] + sizes[ja]

                def dma_a(m, orow):
                    nc.gpsimd.dma_start(
                        out_dram.ap()[m][:, ds(0, h_split)],
                        orow[:, ds(0, h_split)],
                    )

                def dma_b(m, orow):
                    if C > h_split:
                        nc.gpsimd.dma_start(
                            out_dram.ap()[m][:, ds(h_split, C - h_split)],
                            orow[:, ds(h_split, C - h_split)],
                        )

                return ja, dma_a, dma_b

            a_ja, a_dma_a, a_dma_b = make_out_dmas(outa, a_sizes, a_offs, CA)
            b_ja, b_dma_a, b_dma_b = make_out_dmas(outb, b_sizes, b_offs, CB)
            J_A, J_B = len(a_sizes), len(b_sizes)

            # ---- job A startup: first nq rows in k-chunks chasing arrivals.
            # m0: chunks [0,2),[2,4),[4,8); m1/m2: [0,4),[4,8)
            chunks = {0: TA_GROUPS}
            for m in range(1, nq):
                chunks[m] = [(0, 4), (4, 8)]
            acc_tiles = {}
            orow_q = {}
            emit = []  # (m, chunk_index) in token-arrival order
            for tlo, thi in TA_GROUPS:
                for m in range(nq):
                    for qi, (klo, khi) in enumerate(chunks[m]):
                        if tlo < khi <= thi:
                            emit.append((m, qi))
            for m, qi in emit:
                klo, khi = chunks[m][qi]
                last = qi == len(chunks[m]) - 1
                for j in range(J_A):
                    psf = ps_pool.tile([P, 512], f32, tag="ps")
                    pj = psf[:, : a_sizes[j]]
                    for k in range(klo, khi):
                        nc.tensor.matmul(
                            pj,
                            lhs(WA_GROUPS, wa_tiles, m, k),
                            rhs(TA_GROUPS, ta_tiles, a_offs, a_sizes, k, j),
                            start=(k == klo),
                            stop=(k == khi - 1),
                        )
                    if qi == 0:
                        a_full = acc_pool.tile([P, 512], f32, tag=f"acc{m}_{j}")
                        a = a_full[:, : a_sizes[j]]
                        acc_tiles[(m, j)] = a
                        # ACT engine: keeps DVE free during startup
                        nc.scalar.copy(a, pj)
                    elif not last:
                        a = acc_tiles[(m, j)]
                        nc.vector.tensor_add(a, a, pj)
                    else:
                        if m not in orow_q:
                            orow_q[m] = orow_pool.tile(
                                [P, CA], dt, tag="orow", name=f"orow_q{m}"
                            )
                        o = orow_q[m][:, ds(a_offs[j], a_sizes[j])]
                        nc.vector.tensor_add(o, acc_tiles[(m, j)], pj)
                        if j == a_ja:
                            a_dma_a(m, orow_q[m])
                if last:
                    a_dma_b(m, orow_q[m])

            # ---- job A steady rows ----
            for m in range(nq, MT):
                orow = orow_pool.tile([P, CA], dt, tag="orow", name=f"oa{m}")
                for j in range(J_A):
                    psf = ps_pool.tile([P, 512], f32, tag="ps")
                    pj = psf[:, : a_sizes[j]]
                    for k in range(KH):
                        nc.tensor.matmul(
                            pj,
                            lhs(WA_GROUPS, wa_tiles, m, k),
                            rhs(TA_GROUPS, ta_tiles, a_offs, a_sizes, k, j),
                            start=(k == 0),
                            stop=(k == KH - 1),
                        )
                    o = orow[:, ds(a_offs[j], a_sizes[j])]
                    nc.vector.tensor_copy(o, pj)
                    if j == a_ja:
                        a_dma_a(m, orow)
                a_dma_b(m, orow)

            # ---- job B rows (everything resident by now) ----
            for m in range(MT):
                orow = orow_pool.tile([P, CB], dt, tag="orow", name=f"ob{m}")
                for j in range(J_B):
                    psf = ps_pool.tile([P, 512], f32, tag="ps")
                    pj = psf[:, : b_sizes[j]]
                    for k in range(KH):
                        nc.tensor.matmul(
                            pj,
                            lhs(WB_GROUPS, wb_tiles, m, k),
                            rhs(TB_GROUPS, tb_tiles, b_offs, b_sizes, k, j),
                            start=(k == 0),
                            stop=(k == KH - 1),
                        )
                    o = orow[:, ds(b_offs[j], b_sizes[j])]
                    nc.vector.tensor_copy(o, pj)
                    if j == b_ja:
                        b_dma_a(m, orow)
                b_dma_b(m, orow)
    nc.compile()
    return nc


def _get_program(CA, CB):
    key = (CA, CB)
    if key not in _cache:
        _cache[key] = _build(CA, CB)
    return _cache[key]


# ------------------------------------------------------------------- host ---


def kernel(x, y, W_experts, b_experts, W_gate, b_gate):
    import ml_dtypes

    bf16 = np.dtype(ml_dtypes.bfloat16)

    x = np.asarray(x, dtype=np.float32)
    y = np.asarray(y, dtype=np.float32)
    W_experts = np.asarray(W_experts, dtype=np.float32)
    b_experts = np.asarray(b_experts, dtype=np.float32)
    W_gate = np.asarray(W_gate, dtype=np.float32)
    b_gate = np.asarray(b_gate, dtype=np.float32)

    inp = np.concatenate([x, y], axis=1)  # [B, D]

    # ---- routing (host) ----
    logits = inp.astype(np.float64) @ W_gate.T.astype(np.float64) + b_gate
    order = np.argsort(-logits, axis=1, kind="stable")
    top2 = order[:, :TOPK]  # [B, 2]
    v = np.take_along_axis(logits, top2, axis=1)
    v = v - v.max(axis=1, keepdims=True)
    ev = np.exp(v)
    g = (ev / ev.sum(axis=1, keepdims=True)).astype(np.float32)  # [B, 2]

    counts = np.bincount(top2.ravel(), minlength=E)

    idx_list = []
    wgt_list = []
    for e in range(E):
        m0 = top2[:, 0] == e
        m1 = top2[:, 1] == e
        idx_e = np.concatenate([np.nonzero(m0)[0], np.nonzero(m1)[0]])
        w_e = np.concatenate([g[m0, 0], g[m1, 1]])
        idx_list.append(idx_e)
        wgt_list.append(w_e)

    # ---- k-split pairing: 4 biggest experts fill slot A, rest slot B ----
    by_size = np.argsort(-counts, kind="stable")
    big, small = by_size[:4], by_size[4:]
    CA = max(512, int(counts[big[0]]))
    CB = max(512, int(counts[small[0]]))
    # core 2*i   -> (big[i], half 0) + (small[i], half 0)
    # core 2*i+1 -> (big[i], half 1) + (small[i], half 1)
    slots = []  # per core: ((expertA, halfA), (expertB, halfB))
    for i in range(4):
        slots.append(((int(big[i]), 0), (int(small[i]), 0)))
        slots.append(((int(big[i]), 1), (int(small[i]), 1)))

    inp_bf = inp.astype(bf16)  # [B, D]
    w_r = W_experts.reshape(E, MT, P, KT, P)

    def w_half(e, h):
        # [P(ki), MT, KH, P(o)] bf16
        return np.ascontiguousarray(
            w_r[e][:, :, h * KH : (h + 1) * KH, :]
            .transpose(3, 0, 2, 1)
            .astype(bf16)
        )

    tok_cache = {}

    def tok_half(e, h, C):
        key = (e, h)
        if key not in tok_cache:
            sel = inp_bf[idx_list[e]].T.reshape(KT, P, -1)  # [KT, P, n_e]
            tok_cache[key] = sel[h * KH : (h + 1) * KH].transpose(1, 0, 2)
        n_e = len(idx_list[e])
        out = np.zeros((P, KH, C), dtype=bf16)
        out[:, :, :n_e] = tok_cache[key]
        return out

    in_maps = []
    for (ea, ha), (eb, hb) in slots:
        in_maps.append(
            {
                "wa": w_half(ea, ha),
                "ta": tok_half(ea, ha, CA),
                "wb": w_half(eb, hb),
                "tb": tok_half(eb, hb, CB),
            }
        )

    # ---- device ----
    if os.environ.get("BASS_TRACE"):
        _ntff_shim()
    from concourse.bass_utils import run_bass_kernel_spmd

    nc = _get_program(CA, CB)
    res = None
    for attempt in range(3):
        try:
            res = run_bass_kernel_spmd(nc, in_maps, core_ids=list(range(N_CORES)))
            break
        except Exception:
            # the axon-tunneled device occasionally reports a transient
            # NRT_EXEC_UNIT_UNRECOVERABLE; it recovers after a short wait
            if attempt == 2:
                raise
            import time

            time.sleep(20 * (attempt + 1))
            try:
                import jax

                jax.clear_caches()
            except Exception:
                pass
    globals()["_last_res"] = res
    if res.exec_time_ns is not None:
        print(f"HW exec time: {res.exec_time_ns} ns")

    # ---- combine (host): sum the two k-half partials, add bias, apply
    # gate weights, scatter.
    part = {}  # (expert, half) -> [n_e, D] f32
    for core, ((ea, ha), (eb, hb)) in enumerate(slots):
        n_a = len(idx_list[ea])
        part[(ea, ha)] = (
            res.results[core]["outa"].reshape(D, CA)[:, :n_a].T.astype(np.float32)
        )
        n_b = len(idx_list[eb])
        part[(eb, hb)] = (
            res.results[core]["outb"].reshape(D, CB)[:, :n_b].T.astype(np.float32)
        )

    fused = np.zeros((B, D), dtype=np.float32)
    for e in range(E):
        n_e = len(idx_list[e])
        if n_e == 0:
            continue
        rows = part[(e, 0)] + part[(e, 1)] + b_experts[e]
        fused[idx_list[e]] += rows * wgt_list[e][:, None]
    return fused
